# revision 9
# baseline (speedup 1.0000x reference)
"""Distributed multi-head attention (B=2, L=2048, D=4096, H=32) on 8 TRN2 NeuronCores.

Strategy: tensor-parallel over heads (4 heads/core) for QKV+attention, then an
AllToAll that trades head-dims for token-slices so o_proj is token-sharded
(each core computes out[:, its 512 tokens] with the full Wo) — the AllToAll
moves 4 MB/core instead of the 64 MB/core an output AllReduce would.

All matmuls run in bf16 on the TensorEngine (f32 PSUM accumulation).
Host-side prep: transpose/permute/tile weights and x into DMA-friendly
partition-major layouts, pre-cast to bf16. Host post: concatenate the 8
token-shards and transpose.

RoPE trick: Q/K output columns are permuted host-side (per head: even dims
then odd dims, pairs of heads interleaved into 128-row tiles) so the rotation
becomes full-width [128, t] vector ops with no partition-pair shuffles.

Schedule (v2): startup DMAs are split across queues and sliced so the first
QKV matmul starts ~12us in (weights on the scalar ring in first-use order,
x-group tiles filled by 4 slice-DMAs on sync so the kt-chain starts after the
first 1MB).  Attention loops h-outer/batch-inner and the (h0,b0) q/k/v loads
prefetch on the gpsimd ring at g==4 of the QKV phase.  o_proj runs as 4
per-local-head passes (Wo pre-grouped host-side by local head), each pass
ready right after that head's AllToAll, accumulating in SBUF f32 on the
VectorEngine so the post-attention tail is one 67us pass instead of 250us.
"""

import sys

if "/opt/trn_rl_repo" not in sys.path:
    sys.path.insert(0, "/opt/trn_rl_repo")

from contextlib import ExitStack

import ml_dtypes
import numpy as np

import concourse.bass as bass
import concourse.tile as tile
from concourse import bacc, mybir
from concourse import bass_utils

BF16 = mybir.dt.bfloat16
F32 = mybir.dt.float32
NPBF16 = ml_dtypes.bfloat16

NCORES = 8
B, L, D, H, HD = 2, 2048, 4096, 32, 128
T = B * L              # 4096 global tokens
NH = H // NCORES       # 4 heads per core
OC = NH * HD           # 512 projection dims per core
KT = D // 128          # 32 contraction tiles over D
LT = L // 128          # 16 key tiles per batch
TG = 512               # phase-1 token-group width
NG = T // TG           # 8 groups
SH = T // NCORES       # 512 output tokens per core
SHB = SH // B          # 256 per batch
SCALE = 1.0 / float(np.sqrt(HD))

EXP_F = mybir.ActivationFunctionType.Exp


def build_nc():
    nc = bacc.Bacc("TRN2", target_bir_lowering=False, debug=False,
                   num_devices=NCORES)

    # ---- I/O (per-core shards, host-pretiled, bf16) ----
    xT = nc.dram_tensor("xT", [KT, 128, T], BF16, kind="ExternalInput")
    wq = nc.dram_tensor("wq", [128, NH, KT, 128], BF16, kind="ExternalInput")
    wk = nc.dram_tensor("wk", [128, NH, KT, 128], BF16, kind="ExternalInput")
    wv = nc.dram_tensor("wv", [128, KT, OC], BF16, kind="ExternalInput")
    # wo grouped by local head: wo[ot, p, h, j, o] = Wo[ot*128+o, (4j+h)*128+p]
    wo = nc.dram_tensor("wo", [D // 128, 128, NH, NCORES, 128], BF16,
                        kind="ExternalInput")
    cs = nc.dram_tensor("cs", [128, L], F32, kind="ExternalInput")
    sn = nc.dram_tensor("sn", [128, L], F32, kind="ExternalInput")
    ones = nc.dram_tensor("ones", [128, 128], BF16, kind="ExternalInput")
    out = nc.dram_tensor("out", [D, SH], F32, kind="ExternalOutput")

    # ---- internal DRAM (spills + collective bounce) ----
    qsp = [nc.dram_tensor(f"qsp{b}", [NH, 128, L], BF16) for b in range(B)]
    ksp = [nc.dram_tensor(f"ksp{b}", [NH, 128, L], BF16) for b in range(B)]
    vsp = [nc.dram_tensor(f"vsp{b}", [LT, 128, OC], BF16) for b in range(B)]
    # AllToAll split by (batch, head): [shard, HD dims, SHB tokens]
    a2a_in = [[nc.dram_tensor(f"a2ai{b}_{h}", [NCORES, HD, SHB], BF16)
               for h in range(NH)] for b in range(B)]
    a2a_out = [[nc.dram_tensor(f"a2ao{b}_{h}", [NCORES, HD, SHB], BF16)
                for h in range(NH)] for b in range(B)]

    with tile.TileContext(nc) as tc, ExitStack() as ctx:
        singles = ctx.enter_context(tc.tile_pool(name="singles", bufs=1))
        ones_sb = singles.tile([128, 128], BF16, name="ones")
        nc.gpsimd.dma_start(ones_sb[:], ones[:, :])

        # pools that span the phase boundary (prefetch targets)
        qk = ctx.enter_context(tc.tile_pool(name="qk", bufs=2))
        vbp = ctx.enter_context(tc.tile_pool(name="vb", bufs=2))
        vb = [vbp.tile([128, LT, OC], BF16, name="vb") for _ in range(B)]
        q_pre = qk.tile([128, L], BF16, name="q")
        k_pre = qk.tile([128, L], BF16, name="k")

        # ================= Phase 1: QKV projections + RoPE =================
        with ExitStack() as p1:
            wpool = p1.enter_context(tc.tile_pool(name="w", bufs=1))
            # weights stream on the scalar ring in first-use order; wq is
            # filled by two slice-DMAs so pr=0 starts after the first 2MB
            wq_sb = wpool.tile([128, NH, KT, 128], BF16, name="wq")
            for hh in range(NH):
                nc.scalar.dma_start(wq_sb[:, hh:hh + 1, :, :],
                                    wq[:, hh:hh + 1, :, :])
            wk_sb = wpool.tile([128, NH, KT, 128], BF16, name="wk")
            nc.scalar.dma_start(wk_sb[:], wk[:, :, :, :])
            wv_sb = wpool.tile([128, KT, OC], BF16, name="wv")
            nc.scalar.dma_start(wv_sb[:], wv[:, :, :])

            xpool = p1.enter_context(tc.tile_pool(name="xg", bufs=1))
            cpool = p1.enter_context(tc.tile_pool(name="csg", bufs=2))
            tmp = p1.enter_context(tc.tile_pool(name="tmp", bufs=8))
            st = p1.enter_context(tc.tile_pool(name="st", bufs=6))
            ps1 = p1.enter_context(tc.tile_pool(name="ps1", bufs=6, space="PSUM"))

            for g in range(NG):
                b = g // (NG // B)
                pos0 = (g % (NG // B)) * TG          # position within batch
                xg = xpool.tile([128, KT, TG], BF16, name="xg")
                # 4 slice loads, slice 3 first: all kt-chains consume in
                # rotated order [24..31, 0..23] so the slice a chain needs
                # next is always the one freed longest ago (uniform 6us
                # refill slack at group boundaries with bufs=1)
                xsrc = xT[:, :, g * TG:(g + 1) * TG].transpose([1, 0, 2])
                nc.sync.dma_start(xg[:, 24:32, :], xsrc[:, 24:32, :])
                csg = cpool.tile([128, TG], F32, name="csg")
                nc.sync.dma_start(csg[:], cs[:, pos0:pos0 + TG])
                sng = cpool.tile([128, TG], F32, name="sng")
                nc.sync.dma_start(sng[:], sn[:, pos0:pos0 + TG])
                for s in range(3):
                    nc.sync.dma_start(xg[:, 8 * s:8 * (s + 1), :],
                                      xsrc[:, 8 * s:8 * (s + 1), :])

                if g == 4:
                    # b0 spills are complete: prefetch (h0,b0) q/k and v(b0)
                    # on the gpsimd ring so attention starts with no gap
                    nc.gpsimd.dma_start(q_pre[:], qsp[0][0, :, :])
                    nc.gpsimd.dma_start(k_pre[:], ksp[0][0, :, :])
                    nc.gpsimd.dma_start(vb[0][:, :, :],
                                        vsp[0].ap().transpose([1, 0, 2]))

                # Q and K with fused RoPE (kt order rotated, see above)
                kto = list(range(24, 32)) + list(range(24))
                for wsb, sp in ((wq_sb, qsp[b]), (wk_sb, ksp[b])):
                    for pr in range(NH // 2):
                        p_re = ps1.tile([128, TG], F32, name="ps1")
                        p_im = ps1.tile([128, TG], F32, name="ps1")
                        for i, kt in enumerate(kto):
                            nc.tensor.matmul(p_re[:], wsb[:, 2 * pr, kt, :],
                                             xg[:, kt, :],
                                             start=(i == 0), stop=(i == KT - 1))
                        for i, kt in enumerate(kto):
                            nc.tensor.matmul(p_im[:], wsb[:, 2 * pr + 1, kt, :],
                                             xg[:, kt, :],
                                             start=(i == 0), stop=(i == KT - 1))
                        t1 = tmp.tile([128, TG], F32, name="tmp")
                        t2 = tmp.tile([128, TG], F32, name="tmp")
                        t3 = tmp.tile([128, TG], F32, name="tmp")
                        t4 = tmp.tile([128, TG], F32, name="tmp")
                        o_re = st.tile([128, TG], BF16, name="st")
                        o_im = st.tile([128, TG], BF16, name="st")
                        nc.vector.tensor_mul(t1[:], p_re[:], csg[:])
                        nc.vector.tensor_mul(t2[:], p_im[:], sng[:])
                        nc.vector.tensor_sub(o_re[:], t1[:], t2[:])
                        nc.vector.tensor_mul(t3[:], p_re[:], sng[:])
                        nc.vector.tensor_mul(t4[:], p_im[:], csg[:])
                        nc.vector.tensor_add(o_im[:], t3[:], t4[:])
                        ha, hb = 2 * pr, 2 * pr + 1
                        nc.gpsimd.dma_start(sp[ha, 0:64, pos0:pos0 + TG],
                                            o_re[0:64, :])
                        nc.gpsimd.dma_start(sp[hb, 0:64, pos0:pos0 + TG],
                                            o_re[64:128, :])
                        nc.gpsimd.dma_start(sp[ha, 64:128, pos0:pos0 + TG],
                                            o_im[0:64, :])
                        nc.gpsimd.dma_start(sp[hb, 64:128, pos0:pos0 + TG],
                                            o_im[64:128, :])

                # V (layout [t, oc])
                for sub in range(TG // 128):
                    pv = ps1.tile([128, OC], F32, name="ps1")
                    for i, kt in enumerate(kto):
                        nc.tensor.matmul(pv[:], xg[:, kt, sub * 128:(sub + 1) * 128],
                                         wv_sb[:, kt, :],
                                         start=(i == 0), stop=(i == KT - 1))
                    vo = st.tile([128, OC], BF16, name="st")
                    nc.scalar.copy(vo[:], pv[:])
                    tt = pos0 // 128 + sub
                    nc.gpsimd.dma_start(vsp[b][tt, :, :], vo[:])

        # ====== Phase 2+3: attention (h-outer), AllToAll, o_proj passes ======
        with ExitStack() as p2:
            ep = p2.enter_context(tc.tile_pool(name="ep", bufs=4))
            pvc = p2.enter_context(tc.tile_pool(name="pvc", bufs=4))
            trp = p2.enter_context(tc.tile_pool(name="tr", bufs=6))
            rc = p2.enter_context(tc.tile_pool(name="rc", bufs=4))
            ao = p2.enter_context(tc.tile_pool(name="ao", bufs=3))
            rhp = p2.enter_context(tc.tile_pool(name="rh", bufs=1))
            wop = p2.enter_context(tc.tile_pool(name="wo", bufs=3))
            oac = p2.enter_context(tc.tile_pool(name="oac", bufs=1))
            osb = p2.enter_context(tc.tile_pool(name="osb", bufs=4))
            ps_s = p2.enter_context(tc.tile_pool(name="ps_s", bufs=2, space="PSUM"))
            ps_pv = p2.enter_context(tc.tile_pool(name="ps_pv", bufs=2, space="PSUM"))
            ps_o = p2.enter_context(tc.tile_pool(name="ps_o", bufs=2, space="PSUM"))

            # v(b1) spills land at the end of g7; load right away on sync
            nc.sync.dma_start(vb[1][:, :, :], vsp[1].ap().transpose([1, 0, 2]))

            rh = rhp.tile([128, KT, SH], BF16, name="rh")
            rh4 = rh[:].rearrange("p (j f) t -> p j f t", f=4)   # [128,8,4,SH]
            out_acc = oac.tile([128, D // 128, SH], F32, name="oac")

            def attn_slot(h, b):
                    if h == 0 and b == 0:
                        q_sb, k_sb = q_pre, k_pre
                    else:
                        q_sb = qk.tile([128, L], BF16, name="q")
                        nc.scalar.dma_start(q_sb[:], qsp[b][h, :, :])
                        k_sb = qk.tile([128, L], BF16, name="k")
                        nc.scalar.dma_start(k_sb[:], ksp[b][h, :, :])
                    for half in range(2):
                        q0 = half * 1024
                        pvs = [ps_pv.tile([128, 512], F32, name="ps_pv")
                               for _ in range(2)]
                        tree = []          # bf16 pairwise row-sum tree
                        for kt in range(LT):
                            s_ps = ps_s.tile([128, 1024], F32, name="ps_s")
                            nc.tensor.matmul(s_ps[:, 0:512],
                                             k_sb[:, kt * 128:(kt + 1) * 128],
                                             q_sb[:, q0:q0 + 512],
                                             start=True, stop=True)
                            nc.tensor.matmul(s_ps[:, 512:1024],
                                             k_sb[:, kt * 128:(kt + 1) * 128],
                                             q_sb[:, q0 + 512:q0 + 1024],
                                             start=True, stop=True)
                            e_t = ep.tile([128, 1024], BF16, name="ep")
                            nc.scalar.activation(e_t[:], s_ps[:], EXP_F, scale=SCALE)
                            first, last = (kt == 0), (kt == LT - 1)
                            for c in range(2):
                                nc.tensor.matmul(pvs[c][:],
                                                 vb[b][:, kt, h * 128:(h + 1) * 128],
                                                 e_t[:, c * 512:(c + 1) * 512],
                                                 start=first, stop=last)
                            node = (0, e_t)
                            while tree and tree[-1][0] == node[0]:
                                prev = tree.pop()
                                nt = trp.tile([128, 1024], BF16, name="tr")
                                nc.vector.tensor_add(nt[:], prev[1][:], node[1][:])
                                node = (node[0] + 1, nt)
                            tree.append(node)
                        assert len(tree) == 1
                        root = tree[0][1]
                        # drain pv psums to SBUF so next half's MMs start now
                        pvcs = []
                        for c in range(2):
                            pc = pvc.tile([128, 512], F32, name="pvc")
                            nc.vector.tensor_copy(pc[:], pvs[c][:])
                            pvcs.append(pc)
                        # partition-reduce the row-sum tree root (pv slots free)
                        rts = [ps_pv.tile([128, 512], F32, name="ps_pv")
                               for _ in range(2)]
                        for c in range(2):
                            nc.tensor.matmul(rts[c][:], ones_sb[:],
                                             root[:, c * 512:(c + 1) * 512],
                                             start=True, stop=True)
                        for c in range(2):
                            rec = rc.tile([128, 512], F32, name="rc")
                            nc.vector.reciprocal_approx_fast(out=rec[:],
                                                             in_=rts[c][:])
                            at = ao.tile([128, 512], BF16, name="ao")
                            nc.vector.tensor_mul(at[:], pvcs[c][:], rec[:])
                            ci = half * 2 + c
                            nc.gpsimd.dma_start(
                                a2a_in[b][h][2 * ci, :, :], at[:, 0:SHB])
                            nc.gpsimd.dma_start(
                                a2a_in[b][h][2 * ci + 1, :, :], at[:, SHB:2 * SHB])
                    nc.gpsimd.collective_compute(
                        "AllToAll", mybir.AluOpType.bypass,
                        replica_groups=[list(range(NCORES))],
                        ins=[a2a_in[b][h].ap().opt()],
                        outs=[a2a_out[b][h].ap().opt()],
                    )
                    nc.sync.dma_start(
                        rh4[:, :, h, b * SHB:(b + 1) * SHB],
                        a2a_out[b][h].ap().transpose([1, 0, 2]))

            def oproj_pass(h, c0, c1):
                # contract the 8 global heads {4j+h} over token cols [c0,c1)
                n = c1 - c0
                for ot in range(D // 128):
                    wot = wop.tile([128, NCORES, 128], BF16, name="wo")
                    nc.sync.dma_start(wot[:], wo[ot, :, h, :, :])
                    po = ps_o.tile([128, SH], F32, name="ps_o")
                    for j in range(NCORES):
                        nc.tensor.matmul(po[:, 0:n], wot[:, j, :],
                                         rh[:, 4 * j + h, c0:c1],
                                         start=(j == 0), stop=(j == NCORES - 1))
                    if h == 0:
                        nc.vector.tensor_copy(out_acc[:, ot, c0:c1], po[:, 0:n])
                    elif h < NH - 1:
                        nc.vector.tensor_add(out_acc[:, ot, c0:c1], po[:, 0:n],
                                             out_acc[:, ot, c0:c1])
                    else:
                        o_sb = osb.tile([128, SH], F32, name="osb")
                        nc.vector.tensor_add(o_sb[:, 0:n], po[:, 0:n],
                                             out_acc[:, ot, c0:c1])
                        nc.gpsimd.dma_start(out[ot * 128:(ot + 1) * 128, c0:c1],
                                            o_sb[:, 0:n])

            # slot order with one-slot-lagged passes: a pass consumes its
            # head's AllToAlls ~40us after the last one fires, tolerating
            # cross-core skew; pass 3 is split by batch so its b0 half covers
            # the last AllToAll's latency
            attn_slot(0, 0); attn_slot(0, 1); attn_slot(1, 0)
            oproj_pass(0, 0, SH)
            attn_slot(1, 1); attn_slot(2, 0)
            oproj_pass(1, 0, SH)
            attn_slot(2, 1); attn_slot(3, 0)
            oproj_pass(2, 0, SH)
            attn_slot(3, 1)
            oproj_pass(3, 0, SHB)
            oproj_pass(3, SHB, SH)

    nc.compile()
    return nc


def _qk_row_perm():
    # local row order: [h0re|h1re],[h0im|h1im],[h2re|h3re],[h2im|h3im]
    rows = []
    for pr in range(NH // 2):
        ha, hb = 2 * pr, 2 * pr + 1
        rows += [ha * HD + 2 * i for i in range(HD // 2)]
        rows += [hb * HD + 2 * i for i in range(HD // 2)]
        rows += [ha * HD + 2 * i + 1 for i in range(HD // 2)]
        rows += [hb * HD + 2 * i + 1 for i in range(HD // 2)]
    return np.array(rows)


def _prep_inputs(x, freqs_cos, freqs_sin, Wq, Wk, Wv, Wo):
    x = np.asarray(x, np.float32).reshape(T, D)
    Wq, Wk, Wv, Wo = (np.asarray(w, np.float32) for w in (Wq, Wk, Wv, Wo))
    fc = np.asarray(freqs_cos, np.float32)
    fs = np.asarray(freqs_sin, np.float32)

    # shared tensors
    xT = np.ascontiguousarray(
        x.reshape(T, KT, 128).transpose(1, 2, 0)).astype(NPBF16)        # [KT,128,T]
    # wo[ot, p, h, j, o] = Wo[ot*128+o, (4j+h)*128+p]
    woh = np.ascontiguousarray(
        Wo.reshape(D // 128, 128, NCORES, NH, 128).transpose(0, 4, 3, 2, 1)
    ).astype(NPBF16)
    csh = np.ascontiguousarray(np.concatenate([fc.T, fc.T], 0))          # [128,L]
    snh = np.ascontiguousarray(np.concatenate([fs.T, fs.T], 0))
    ones = np.ones([128, 128], NPBF16)

    perm = _qk_row_perm()
    in_maps = []
    for i in range(NCORES):
        rows = slice(OC * i, OC * (i + 1))
        wqi = Wq[rows][perm]                                             # [512, D]
        wki = Wk[rows][perm]
        wqh = np.ascontiguousarray(
            wqi.reshape(NH, 128, KT, 128).transpose(3, 0, 2, 1)).astype(NPBF16)
        wkh = np.ascontiguousarray(
            wki.reshape(NH, 128, KT, 128).transpose(3, 0, 2, 1)).astype(NPBF16)
        wvh = np.ascontiguousarray(
            Wv[rows].reshape(OC, KT, 128).transpose(2, 1, 0)).astype(NPBF16)
        in_maps.append({
            "xT": xT, "wq": wqh, "wk": wkh, "wv": wvh, "wo": woh,
            "cs": csh, "sn": snh, "ones": ones,
        })
    return in_maps


_NC_CACHE = None


def _get_nc():
    global _NC_CACHE
    if _NC_CACHE is None:
        _NC_CACHE = build_nc()
    return _NC_CACHE


def _run(in_maps, trace=False):
    nc = _get_nc()
    res = bass_utils.run_bass_kernel_spmd(
        nc, in_maps, core_ids=list(range(NCORES)), trace=trace)
    return res


def _assemble(results):
    out = np.empty((B, L, D), np.float32)
    for i in range(NCORES):
        o = results[i]["out"]                       # [D, SH] f32
        for b in range(B):
            out[b, SHB * i:SHB * (i + 1), :] = o[:, b * SHB:(b + 1) * SHB].T
    return out


def kernel(x, freqs_cos, freqs_sin, Wq, Wk, Wv, Wo):
    in_maps = _prep_inputs(x, freqs_cos, freqs_sin, Wq, Wk, Wv, Wo)
    res = _run(in_maps, trace=False)
    return _assemble(res.results)


# revision 13
# speedup vs baseline: 1.0688x; 1.0688x over previous
"""Distributed multi-head attention (B=2, L=2048, D=4096, H=32) on 8 TRN2 NeuronCores.

Strategy: tensor-parallel over heads (4 heads/core) for QKV+attention, then an
AllToAll that trades head-dims for token-slices so o_proj is token-sharded
(each core computes out[:, its 512 tokens] with the full Wo) — the AllToAll
moves 4 MB/core instead of the 64 MB/core an output AllReduce would.

All matmuls run in bf16 on the TensorEngine (f32 PSUM accumulation).
Host-side prep: transpose/permute/tile weights and x into DMA-friendly
partition-major layouts, pre-cast to bf16. Host post: concatenate the 8
token-shards and transpose.

RoPE trick: Q/K output columns are permuted host-side (per head: even dims
then odd dims, pairs of heads interleaved into 128-row tiles) so the rotation
becomes full-width [128, t] vector ops with no partition-pair shuffles.

Schedule (v2): startup DMAs are split across queues and sliced so the first
QKV matmul starts ~12us in (weights on the scalar ring in first-use order,
x-group tiles filled by 4 slice-DMAs on sync so the kt-chain starts after the
first 1MB).  Attention loops h-outer/batch-inner and the (h0,b0) q/k/v loads
prefetch on the gpsimd ring at g==4 of the QKV phase.  o_proj runs as 4
per-local-head passes (Wo pre-grouped host-side by local head), each pass
ready right after that head's AllToAll, accumulating in SBUF f32 on the
VectorEngine so the post-attention tail is one 67us pass instead of 250us.
"""

import sys

if "/opt/trn_rl_repo" not in sys.path:
    sys.path.insert(0, "/opt/trn_rl_repo")

from contextlib import ExitStack

import ml_dtypes
import numpy as np

import concourse.bass as bass
import concourse.tile as tile
from concourse import bacc, mybir
from concourse import bass_utils

BF16 = mybir.dt.bfloat16
F32 = mybir.dt.float32
NPBF16 = ml_dtypes.bfloat16

NCORES = 8
B, L, D, H, HD = 2, 2048, 4096, 32, 128
T = B * L              # 4096 global tokens
NH = H // NCORES       # 4 heads per core
OC = NH * HD           # 512 projection dims per core
KT = D // 128          # 32 contraction tiles over D
LT = L // 128          # 16 key tiles per batch
TG = 512               # phase-1 token-group width
NG = T // TG           # 8 groups
SH = T // NCORES       # 512 output tokens per core
SHB = SH // B          # 256 per batch
SCALE = 1.0 / float(np.sqrt(HD))

EXP_F = mybir.ActivationFunctionType.Exp


def build_nc():
    nc = bacc.Bacc("TRN2", target_bir_lowering=False, debug=False,
                   num_devices=NCORES)

    # ---- I/O (per-core shards, host-pretiled, bf16) ----
    xT = nc.dram_tensor("xT", [KT, 128, T], BF16, kind="ExternalInput")
    wq = nc.dram_tensor("wq", [128, NH, KT, 128], BF16, kind="ExternalInput")
    wk = nc.dram_tensor("wk", [128, NH, KT, 128], BF16, kind="ExternalInput")
    wv = nc.dram_tensor("wv", [128, KT, OC], BF16, kind="ExternalInput")
    # wo grouped by local head: wo[ot, p, h, j, o] = Wo[ot*128+o, (4j+h)*128+p]
    wo = nc.dram_tensor("wo", [D // 128, 128, NH, NCORES, 128], BF16,
                        kind="ExternalInput")
    cs = nc.dram_tensor("cs", [128, L], F32, kind="ExternalInput")
    sn = nc.dram_tensor("sn", [128, L], F32, kind="ExternalInput")
    ones = nc.dram_tensor("ones", [128, 128], BF16, kind="ExternalInput")
    out = nc.dram_tensor("out", [D, SH], F32, kind="ExternalOutput")

    # ---- internal DRAM (spills + collective bounce) ----
    qsp = [nc.dram_tensor(f"qsp{b}", [NH, 128, L], BF16) for b in range(B)]
    ksp = [nc.dram_tensor(f"ksp{b}", [NH, 128, L], BF16) for b in range(B)]
    vsp = [nc.dram_tensor(f"vsp{b}", [LT, 128, OC], BF16) for b in range(B)]
    # AllToAll split by (batch, head): [shard, HD dims, SHB tokens]
    a2a_in = [[nc.dram_tensor(f"a2ai{b}_{h}", [NCORES, HD, SHB], BF16)
               for h in range(NH)] for b in range(B)]
    a2a_out = [[nc.dram_tensor(f"a2ao{b}_{h}", [NCORES, HD, SHB], BF16)
                for h in range(NH)] for b in range(B)]

    with tile.TileContext(nc) as tc, ExitStack() as ctx:
        singles = ctx.enter_context(tc.tile_pool(name="singles", bufs=1))
        ones_sb = singles.tile([128, 128], BF16, name="ones")
        nc.gpsimd.dma_start(ones_sb[:], ones[:, :])

        # pools that span the phase boundary (prefetch targets)
        qk = ctx.enter_context(tc.tile_pool(name="qk", bufs=2))
        vbp = ctx.enter_context(tc.tile_pool(name="vb", bufs=2))
        vb = [vbp.tile([128, LT, OC], BF16, name="vb") for _ in range(B)]
        q_pre = qk.tile([128, L], BF16, name="q")
        k_pre = qk.tile([128, L], BF16, name="k")

        # ================= Phase 1: QKV projections + RoPE =================
        with ExitStack() as p1:
            wpool = p1.enter_context(tc.tile_pool(name="w", bufs=1))
            # weights stream on the scalar ring in first-use order; wq is
            # filled by two slice-DMAs so pr=0 starts after the first 2MB
            wq_sb = wpool.tile([128, NH, KT, 128], BF16, name="wq")
            for hh in range(NH):
                nc.scalar.dma_start(wq_sb[:, hh:hh + 1, :, :],
                                    wq[:, hh:hh + 1, :, :])
            wk_sb = wpool.tile([128, NH, KT, 128], BF16, name="wk")
            nc.scalar.dma_start(wk_sb[:], wk[:, :, :, :])
            wv_sb = wpool.tile([128, KT, OC], BF16, name="wv")
            nc.scalar.dma_start(wv_sb[:], wv[:, :, :])

            xpool = p1.enter_context(tc.tile_pool(name="xg", bufs=1))
            cpool = p1.enter_context(tc.tile_pool(name="csg", bufs=2))
            tmp = p1.enter_context(tc.tile_pool(name="tmp", bufs=8))
            st = p1.enter_context(tc.tile_pool(name="st", bufs=6))
            ps1 = p1.enter_context(tc.tile_pool(name="ps1", bufs=6, space="PSUM"))

            for g in range(NG):
                b = g // (NG // B)
                pos0 = (g % (NG // B)) * TG          # position within batch
                xg = xpool.tile([128, KT, TG], BF16, name="xg")
                # 4 slice loads, slice 3 first: all kt-chains consume in
                # rotated order [24..31, 0..23] so the slice a chain needs
                # next is always the one freed longest ago (uniform 6us
                # refill slack at group boundaries with bufs=1)
                xsrc = xT[:, :, g * TG:(g + 1) * TG].transpose([1, 0, 2])
                nc.sync.dma_start(xg[:, 24:32, :], xsrc[:, 24:32, :])
                # cos/sin ride the scalar ring: keeps the sync ring's DMA
                # counter (which the first matmul of the group waits on)
                # covering only the x slices
                csg = cpool.tile([128, TG], F32, name="csg")
                nc.scalar.dma_start(csg[:], cs[:, pos0:pos0 + TG])
                sng = cpool.tile([128, TG], F32, name="sng")
                nc.scalar.dma_start(sng[:], sn[:, pos0:pos0 + TG])
                for s in range(3):
                    nc.sync.dma_start(xg[:, 8 * s:8 * (s + 1), :],
                                      xsrc[:, 8 * s:8 * (s + 1), :])

                if g == 4:
                    # b0 spills are complete: prefetch (h0,b0) q/k and v(b0)
                    # on the gpsimd ring so attention starts with no gap
                    nc.gpsimd.dma_start(q_pre[:], qsp[0][0, :, :])
                    nc.gpsimd.dma_start(k_pre[:], ksp[0][0, :, :])
                    nc.gpsimd.dma_start(vb[0][:, :, :],
                                        vsp[0].ap().transpose([1, 0, 2]))

                # Q and K with fused RoPE (kt order rotated, see above)
                kto = list(range(24, 32)) + list(range(24))
                for wsb, sp in ((wq_sb, qsp[b]), (wk_sb, ksp[b])):
                    for pr in range(NH // 2):
                        p_re = ps1.tile([128, TG], F32, name="ps1")
                        p_im = ps1.tile([128, TG], F32, name="ps1")
                        for i, kt in enumerate(kto):
                            nc.tensor.matmul(p_re[:], wsb[:, 2 * pr, kt, :],
                                             xg[:, kt, :],
                                             start=(i == 0), stop=(i == KT - 1))
                        for i, kt in enumerate(kto):
                            nc.tensor.matmul(p_im[:], wsb[:, 2 * pr + 1, kt, :],
                                             xg[:, kt, :],
                                             start=(i == 0), stop=(i == KT - 1))
                        t1 = tmp.tile([128, TG], F32, name="tmp")
                        t2 = tmp.tile([128, TG], F32, name="tmp")
                        t3 = tmp.tile([128, TG], F32, name="tmp")
                        t4 = tmp.tile([128, TG], F32, name="tmp")
                        o_re = st.tile([128, TG], BF16, name="st")
                        o_im = st.tile([128, TG], BF16, name="st")
                        nc.vector.tensor_mul(t1[:], p_re[:], csg[:])
                        nc.vector.tensor_mul(t2[:], p_im[:], sng[:])
                        nc.vector.tensor_sub(o_re[:], t1[:], t2[:])
                        nc.vector.tensor_mul(t3[:], p_re[:], sng[:])
                        nc.vector.tensor_mul(t4[:], p_im[:], csg[:])
                        nc.vector.tensor_add(o_im[:], t3[:], t4[:])
                        ha, hb = 2 * pr, 2 * pr + 1
                        nc.gpsimd.dma_start(sp[ha, 0:64, pos0:pos0 + TG],
                                            o_re[0:64, :])
                        nc.gpsimd.dma_start(sp[hb, 0:64, pos0:pos0 + TG],
                                            o_re[64:128, :])
                        nc.gpsimd.dma_start(sp[ha, 64:128, pos0:pos0 + TG],
                                            o_im[0:64, :])
                        nc.gpsimd.dma_start(sp[hb, 64:128, pos0:pos0 + TG],
                                            o_im[64:128, :])

                # V (layout [t, oc])
                for sub in range(TG // 128):
                    pv = ps1.tile([128, OC], F32, name="ps1")
                    for i, kt in enumerate(kto):
                        nc.tensor.matmul(pv[:], xg[:, kt, sub * 128:(sub + 1) * 128],
                                         wv_sb[:, kt, :],
                                         start=(i == 0), stop=(i == KT - 1))
                    vo = st.tile([128, OC], BF16, name="st")
                    nc.scalar.copy(vo[:], pv[:])
                    tt = pos0 // 128 + sub
                    nc.gpsimd.dma_start(vsp[b][tt, :, :], vo[:])

        # ====== Phase 2+3: attention (h-outer), AllToAll, o_proj passes ======
        with ExitStack() as p2:
            ep = p2.enter_context(tc.tile_pool(name="ep", bufs=4))
            pvc = p2.enter_context(tc.tile_pool(name="pvc", bufs=4))
            trp = p2.enter_context(tc.tile_pool(name="tr", bufs=6))
            rc = p2.enter_context(tc.tile_pool(name="rc", bufs=4))
            ao = p2.enter_context(tc.tile_pool(name="ao", bufs=3))
            rhp = p2.enter_context(tc.tile_pool(name="rh", bufs=1))
            wop = p2.enter_context(tc.tile_pool(name="wo", bufs=6))
            oac = p2.enter_context(tc.tile_pool(name="oac", bufs=1))
            osb = p2.enter_context(tc.tile_pool(name="osb", bufs=4))
            ps_s = p2.enter_context(tc.tile_pool(name="ps_s", bufs=2, space="PSUM"))
            ps_pv = p2.enter_context(tc.tile_pool(name="ps_pv", bufs=2, space="PSUM"))
            ps_o = p2.enter_context(tc.tile_pool(name="ps_o", bufs=2, space="PSUM"))

            # v(b1) spills land at the end of g7; load right away on sync
            nc.sync.dma_start(vb[1][:, :, :], vsp[1].ap().transpose([1, 0, 2]))

            rh = rhp.tile([128, KT, SH], BF16, name="rh")
            rh4 = rh[:].rearrange("p (j f) t -> p j f t", f=4)   # [128,8,4,SH]
            out_acc = oac.tile([128, D // 128, SH], F32, name="oac")

            def attn_slot(h, b):
                    if h == 0 and b == 0:
                        q_sb, k_sb = q_pre, k_pre
                    else:
                        q_sb = qk.tile([128, L], BF16, name="q")
                        nc.scalar.dma_start(q_sb[:], qsp[b][h, :, :])
                        k_sb = qk.tile([128, L], BF16, name="k")
                        nc.scalar.dma_start(k_sb[:], ksp[b][h, :, :])
                    for half in range(2):
                        q0 = half * 1024
                        pvs = [ps_pv.tile([128, 512], F32, name="ps_pv")
                               for _ in range(2)]
                        tree = []          # bf16 pairwise row-sum tree
                        for kt in range(LT):
                            s_ps = ps_s.tile([128, 1024], F32, name="ps_s")
                            nc.tensor.matmul(s_ps[:, 0:512],
                                             k_sb[:, kt * 128:(kt + 1) * 128],
                                             q_sb[:, q0:q0 + 512],
                                             start=True, stop=True)
                            nc.tensor.matmul(s_ps[:, 512:1024],
                                             k_sb[:, kt * 128:(kt + 1) * 128],
                                             q_sb[:, q0 + 512:q0 + 1024],
                                             start=True, stop=True)
                            e_t = ep.tile([128, 1024], BF16, name="ep")
                            nc.scalar.activation(e_t[:], s_ps[:], EXP_F, scale=SCALE)
                            first, last = (kt == 0), (kt == LT - 1)
                            for c in range(2):
                                nc.tensor.matmul(pvs[c][:],
                                                 vb[b][:, kt, h * 128:(h + 1) * 128],
                                                 e_t[:, c * 512:(c + 1) * 512],
                                                 start=first, stop=last)
                            node = (0, e_t)
                            while tree and tree[-1][0] == node[0]:
                                prev = tree.pop()
                                nt = trp.tile([128, 1024], BF16, name="tr")
                                nc.vector.tensor_add(nt[:], prev[1][:], node[1][:])
                                node = (node[0] + 1, nt)
                            tree.append(node)
                        assert len(tree) == 1
                        root = tree[0][1]
                        # drain pv psums to SBUF so next half's MMs start now
                        pvcs = []
                        for c in range(2):
                            pc = pvc.tile([128, 512], F32, name="pvc")
                            nc.vector.tensor_copy(pc[:], pvs[c][:])
                            pvcs.append(pc)
                        # partition-reduce the row-sum tree root (pv slots free)
                        rts = [ps_pv.tile([128, 512], F32, name="ps_pv")
                               for _ in range(2)]
                        for c in range(2):
                            nc.tensor.matmul(rts[c][:], ones_sb[:],
                                             root[:, c * 512:(c + 1) * 512],
                                             start=True, stop=True)
                        for c in range(2):
                            rec = rc.tile([128, 512], F32, name="rc")
                            nc.vector.reciprocal_approx_fast(out=rec[:],
                                                             in_=rts[c][:])
                            at = ao.tile([128, 512], BF16, name="ao")
                            nc.vector.tensor_mul(at[:], pvcs[c][:], rec[:])
                            ci = half * 2 + c
                            nc.gpsimd.dma_start(
                                a2a_in[b][h][2 * ci, :, :], at[:, 0:SHB])
                            nc.gpsimd.dma_start(
                                a2a_in[b][h][2 * ci + 1, :, :], at[:, SHB:2 * SHB])
                    nc.gpsimd.collective_compute(
                        "AllToAll", mybir.AluOpType.bypass,
                        replica_groups=[list(range(NCORES))],
                        ins=[a2a_in[b][h].ap().opt()],
                        outs=[a2a_out[b][h].ap().opt()],
                    )
                    nc.sync.dma_start(
                        rh4[:, :, h, b * SHB:(b + 1) * SHB],
                        a2a_out[b][h].ap().transpose([1, 0, 2]))

            def oproj_pass(h, c0, c1):
                # contract the 8 global heads {4j+h} over token cols [c0,c1)
                n = c1 - c0
                for ot in range(D // 128):
                    wot = wop.tile([128, NCORES, 128], BF16, name="wo")
                    nc.sync.dma_start(wot[:], wo[ot, :, h, :, :])
                    po = ps_o.tile([128, SH], F32, name="ps_o")
                    for j in range(NCORES):
                        nc.tensor.matmul(po[:, 0:n], wot[:, j, :],
                                         rh[:, 4 * j + h, c0:c1],
                                         start=(j == 0), stop=(j == NCORES - 1))
                    if h == 0:
                        nc.vector.tensor_copy(out_acc[:, ot, c0:c1], po[:, 0:n])
                    elif h < NH - 1:
                        nc.vector.tensor_add(out_acc[:, ot, c0:c1], po[:, 0:n],
                                             out_acc[:, ot, c0:c1])
                    else:
                        o_sb = osb.tile([128, SH], F32, name="osb")
                        nc.vector.tensor_add(o_sb[:, 0:n], po[:, 0:n],
                                             out_acc[:, ot, c0:c1])
                        nc.gpsimd.dma_start(out[ot * 128:(ot + 1) * 128, c0:c1],
                                            o_sb[:, 0:n])

            # slot order with lagged passes: each pass consumes its head's
            # AllToAlls at least one b1-slot (~40us) after the last one
            # fires, tolerating cross-core skew; P2 is held until after the
            # final slot so its 67us of matmuls cover the last AllToAll's
            # latency before P3 needs it
            attn_slot(0, 0); attn_slot(0, 1); attn_slot(1, 0); attn_slot(1, 1)
            oproj_pass(0, 0, SH)
            attn_slot(2, 0); attn_slot(2, 1)
            oproj_pass(1, 0, SH)
            attn_slot(3, 0); attn_slot(3, 1)
            oproj_pass(2, 0, SH)
            oproj_pass(3, 0, SH)

    nc.compile()
    return nc


def _qk_row_perm():
    # local row order: [h0re|h1re],[h0im|h1im],[h2re|h3re],[h2im|h3im]
    rows = []
    for pr in range(NH // 2):
        ha, hb = 2 * pr, 2 * pr + 1
        rows += [ha * HD + 2 * i for i in range(HD // 2)]
        rows += [hb * HD + 2 * i for i in range(HD // 2)]
        rows += [ha * HD + 2 * i + 1 for i in range(HD // 2)]
        rows += [hb * HD + 2 * i + 1 for i in range(HD // 2)]
    return np.array(rows)


def _prep_inputs(x, freqs_cos, freqs_sin, Wq, Wk, Wv, Wo):
    x = np.asarray(x, np.float32).reshape(T, D)
    Wq, Wk, Wv, Wo = (np.asarray(w, np.float32) for w in (Wq, Wk, Wv, Wo))
    fc = np.asarray(freqs_cos, np.float32)
    fs = np.asarray(freqs_sin, np.float32)

    # shared tensors
    xT = np.ascontiguousarray(
        x.reshape(T, KT, 128).transpose(1, 2, 0)).astype(NPBF16)        # [KT,128,T]
    # wo[ot, p, h, j, o] = Wo[ot*128+o, (4j+h)*128+p]
    woh = np.ascontiguousarray(
        Wo.reshape(D // 128, 128, NCORES, NH, 128).transpose(0, 4, 3, 2, 1)
    ).astype(NPBF16)
    csh = np.ascontiguousarray(np.concatenate([fc.T, fc.T], 0))          # [128,L]
    snh = np.ascontiguousarray(np.concatenate([fs.T, fs.T], 0))
    ones = np.ones([128, 128], NPBF16)

    perm = _qk_row_perm()
    in_maps = []
    for i in range(NCORES):
        rows = slice(OC * i, OC * (i + 1))
        wqi = Wq[rows][perm]                                             # [512, D]
        wki = Wk[rows][perm]
        wqh = np.ascontiguousarray(
            wqi.reshape(NH, 128, KT, 128).transpose(3, 0, 2, 1)).astype(NPBF16)
        wkh = np.ascontiguousarray(
            wki.reshape(NH, 128, KT, 128).transpose(3, 0, 2, 1)).astype(NPBF16)
        wvh = np.ascontiguousarray(
            Wv[rows].reshape(OC, KT, 128).transpose(2, 1, 0)).astype(NPBF16)
        in_maps.append({
            "xT": xT, "wq": wqh, "wk": wkh, "wv": wvh, "wo": woh,
            "cs": csh, "sn": snh, "ones": ones,
        })
    return in_maps


_NC_CACHE = None


def _get_nc():
    global _NC_CACHE
    if _NC_CACHE is None:
        _NC_CACHE = build_nc()
    return _NC_CACHE


def _run(in_maps, trace=False):
    nc = _get_nc()
    res = bass_utils.run_bass_kernel_spmd(
        nc, in_maps, core_ids=list(range(NCORES)), trace=trace)
    return res


def _assemble(results):
    out = np.empty((B, L, D), np.float32)
    for i in range(NCORES):
        o = results[i]["out"]                       # [D, SH] f32
        for b in range(B):
            out[b, SHB * i:SHB * (i + 1), :] = o[:, b * SHB:(b + 1) * SHB].T
    return out


def kernel(x, freqs_cos, freqs_sin, Wq, Wk, Wv, Wo):
    in_maps = _prep_inputs(x, freqs_cos, freqs_sin, Wq, Wk, Wv, Wo)
    res = _run(in_maps, trace=False)
    return _assemble(res.results)


# revision 19
# speedup vs baseline: 1.1145x; 1.0428x over previous
"""Distributed multi-head attention (B=2, L=2048, D=4096, H=32) on 8 TRN2 NeuronCores.

Strategy: tensor-parallel over heads (4 heads/core) for QKV+attention, then an
AllToAll that trades head-dims for token-slices so o_proj is token-sharded
(each core computes out[:, its 512 tokens] with the full Wo) — the AllToAll
moves 4 MB/core instead of the 64 MB/core an output AllReduce would.

All matmuls run in bf16 on the TensorEngine (f32 PSUM accumulation).
Host-side prep: transpose/permute/tile weights and x into DMA-friendly
partition-major layouts, pre-cast to bf16. Host post: concatenate the 8
token-shards and transpose.

RoPE trick: Q/K output columns are permuted host-side (per head: even dims
then odd dims, pairs of heads interleaved into 128-row tiles) so the rotation
becomes full-width [128, t] vector ops with no partition-pair shuffles.

Schedule (v2): startup DMAs are split across queues and sliced so the first
QKV matmul starts ~12us in (weights on the scalar ring in first-use order,
x-group tiles filled by 4 slice-DMAs on sync so the kt-chain starts after the
first 1MB).  Attention loops h-outer/batch-inner and the (h0,b0) q/k/v loads
prefetch on the gpsimd ring at g==4 of the QKV phase.  o_proj runs as 4
per-local-head passes (Wo pre-grouped host-side by local head), each pass
ready right after that head's AllToAll, accumulating in SBUF f32 on the
VectorEngine so the post-attention tail is one 67us pass instead of 250us.
"""

import sys

if "/opt/trn_rl_repo" not in sys.path:
    sys.path.insert(0, "/opt/trn_rl_repo")

from contextlib import ExitStack

import ml_dtypes
import numpy as np

import concourse.bass as bass
import concourse.tile as tile
from concourse import bacc, mybir
from concourse import bass_utils

BF16 = mybir.dt.bfloat16
F32 = mybir.dt.float32
NPBF16 = ml_dtypes.bfloat16

NCORES = 8
B, L, D, H, HD = 2, 2048, 4096, 32, 128
T = B * L              # 4096 global tokens
NH = H // NCORES       # 4 heads per core
OC = NH * HD           # 512 projection dims per core
KT = D // 128          # 32 contraction tiles over D
LT = L // 128          # 16 key tiles per batch
TG = 512               # phase-1 token-group width
NG = T // TG           # 8 groups
SH = T // NCORES       # 512 output tokens per core
SHB = SH // B          # 256 per batch
SCALE = 1.0 / float(np.sqrt(HD))

EXP_F = mybir.ActivationFunctionType.Exp


def build_nc():
    nc = bacc.Bacc("TRN2", target_bir_lowering=False, debug=False,
                   num_devices=NCORES)

    # ---- I/O (per-core shards, host-pretiled, bf16) ----
    xT = nc.dram_tensor("xT", [KT, 128, T], BF16, kind="ExternalInput")
    wq = nc.dram_tensor("wq", [128, NH, KT, 128], BF16, kind="ExternalInput")
    wk = nc.dram_tensor("wk", [128, NH, KT, 128], BF16, kind="ExternalInput")
    wv = nc.dram_tensor("wv", [128, KT, OC], BF16, kind="ExternalInput")
    # wo grouped by local head: wo[ot, p, h, j, o] = Wo[ot*128+o, (4j+h)*128+p]
    wo = nc.dram_tensor("wo", [D // 128, 128, NH, NCORES, 128], BF16,
                        kind="ExternalInput")
    cs = nc.dram_tensor("cs", [128, L], F32, kind="ExternalInput")
    sn = nc.dram_tensor("sn", [128, L], F32, kind="ExternalInput")
    ones = nc.dram_tensor("ones", [128, 128], BF16, kind="ExternalInput")
    out = nc.dram_tensor("out", [D, SH], F32, kind="ExternalOutput")

    # ---- internal DRAM (spills + collective bounce) ----
    qsp = [nc.dram_tensor(f"qsp{b}", [NH, 128, L], BF16) for b in range(B)]
    ksp = [nc.dram_tensor(f"ksp{b}", [NH, 128, L], BF16) for b in range(B)]
    vsp = [nc.dram_tensor(f"vsp{b}", [LT, 128, OC], BF16) for b in range(B)]
    # AllToAll split by (batch, head): [shard, HD dims, SHB tokens]
    a2a_in = [[nc.dram_tensor(f"a2ai{b}_{h}", [NCORES, HD, SHB], BF16)
               for h in range(NH)] for b in range(B)]
    a2a_out = [[nc.dram_tensor(f"a2ao{b}_{h}", [NCORES, HD, SHB], BF16)
                for h in range(NH)] for b in range(B)]

    with tile.TileContext(nc) as tc, ExitStack() as ctx:
        singles = ctx.enter_context(tc.tile_pool(name="singles", bufs=1))
        ones_sb = singles.tile([128, 128], BF16, name="ones")
        nc.gpsimd.dma_start(ones_sb[:], ones[:, :])

        # pools that span the phase boundary (prefetch targets)
        qk = ctx.enter_context(tc.tile_pool(name="qk", bufs=2))
        q_pre = qk.tile([128, L], BF16, name="q")
        k_pre = qk.tile([128, L], BF16, name="k")

        # ================= Phase 1: QKV projections + RoPE =================
        with ExitStack() as p1:
            wpool = p1.enter_context(tc.tile_pool(name="w", bufs=1))
            # weights stream on the scalar ring in first-use order; wq is
            # filled by two slice-DMAs so pr=0 starts after the first 2MB
            wq_sb = wpool.tile([128, NH, KT, 128], BF16, name="wq")
            for hh in range(NH):
                nc.scalar.dma_start(wq_sb[:, hh:hh + 1, :, :],
                                    wq[:, hh:hh + 1, :, :])
            wk_sb = wpool.tile([128, NH, KT, 128], BF16, name="wk")
            nc.scalar.dma_start(wk_sb[:], wk[:, :, :, :])
            wv_sb = wpool.tile([128, KT, OC], BF16, name="wv")
            nc.scalar.dma_start(wv_sb[:], wv[:, :, :])

            xpool = p1.enter_context(tc.tile_pool(name="xg", bufs=2))
            cpool = p1.enter_context(tc.tile_pool(name="csg", bufs=2))
            tmp = p1.enter_context(tc.tile_pool(name="tmp", bufs=8))
            st = p1.enter_context(tc.tile_pool(name="st", bufs=6))
            ps1 = p1.enter_context(tc.tile_pool(name="ps1", bufs=6, space="PSUM"))

            for g in range(NG):
                b = g // (NG // B)
                pos0 = (g % (NG // B)) * TG          # position within batch
                xg = xpool.tile([128, KT, TG], BF16, name="xg")
                # 4 slice loads, slice 3 first: all kt-chains consume in
                # rotated order [24..31, 0..23] so the slice a chain needs
                # next is always the one freed longest ago (uniform 6us
                # refill slack at group boundaries with bufs=1)
                xsrc = xT[:, :, g * TG:(g + 1) * TG].transpose([1, 0, 2])
                nc.sync.dma_start(xg[:, 24:32, :], xsrc[:, 24:32, :])
                # cos/sin ride the scalar ring: keeps the sync ring's DMA
                # counter (which the first matmul of the group waits on)
                # covering only the x slices
                csg = cpool.tile([128, TG], F32, name="csg")
                nc.scalar.dma_start(csg[:], cs[:, pos0:pos0 + TG])
                sng = cpool.tile([128, TG], F32, name="sng")
                nc.scalar.dma_start(sng[:], sn[:, pos0:pos0 + TG])
                for s in range(3):
                    nc.sync.dma_start(xg[:, 8 * s:8 * (s + 1), :],
                                      xsrc[:, 8 * s:8 * (s + 1), :])

                if g == 4:
                    # b0 spills are complete: prefetch (h0,b0) q/k on the
                    # gpsimd ring so attention starts with no gap
                    nc.gpsimd.dma_start(q_pre[:], qsp[0][0, :, :])
                    nc.gpsimd.dma_start(k_pre[:], ksp[0][0, :, :])

                # Q and K with fused RoPE (kt order rotated, see above)
                kto = list(range(24, 32)) + list(range(24))
                for wsb, sp in ((wq_sb, qsp[b]), (wk_sb, ksp[b])):
                    for pr in range(NH // 2):
                        p_re = ps1.tile([128, TG], F32, name="ps1")
                        p_im = ps1.tile([128, TG], F32, name="ps1")
                        for i, kt in enumerate(kto):
                            nc.tensor.matmul(p_re[:], wsb[:, 2 * pr, kt, :],
                                             xg[:, kt, :],
                                             start=(i == 0), stop=(i == KT - 1))
                        for i, kt in enumerate(kto):
                            nc.tensor.matmul(p_im[:], wsb[:, 2 * pr + 1, kt, :],
                                             xg[:, kt, :],
                                             start=(i == 0), stop=(i == KT - 1))
                        t1 = tmp.tile([128, TG], F32, name="tmp")
                        t2 = tmp.tile([128, TG], F32, name="tmp")
                        t3 = tmp.tile([128, TG], F32, name="tmp")
                        t4 = tmp.tile([128, TG], F32, name="tmp")
                        o_re = st.tile([128, TG], BF16, name="st")
                        o_im = st.tile([128, TG], BF16, name="st")
                        nc.vector.tensor_mul(t1[:], p_re[:], csg[:])
                        nc.vector.tensor_mul(t2[:], p_im[:], sng[:])
                        nc.vector.tensor_sub(o_re[:], t1[:], t2[:])
                        nc.vector.tensor_mul(t3[:], p_re[:], sng[:])
                        nc.vector.tensor_mul(t4[:], p_im[:], csg[:])
                        nc.vector.tensor_add(o_im[:], t3[:], t4[:])
                        ha, hb = 2 * pr, 2 * pr + 1
                        nc.gpsimd.dma_start(sp[ha, 0:64, pos0:pos0 + TG],
                                            o_re[0:64, :])
                        nc.gpsimd.dma_start(sp[hb, 0:64, pos0:pos0 + TG],
                                            o_re[64:128, :])
                        nc.gpsimd.dma_start(sp[ha, 64:128, pos0:pos0 + TG],
                                            o_im[0:64, :])
                        nc.gpsimd.dma_start(sp[hb, 64:128, pos0:pos0 + TG],
                                            o_im[64:128, :])

                # V (layout [t, oc])
                for sub in range(TG // 128):
                    pv = ps1.tile([128, OC], F32, name="ps1")
                    for i, kt in enumerate(kto):
                        nc.tensor.matmul(pv[:], xg[:, kt, sub * 128:(sub + 1) * 128],
                                         wv_sb[:, kt, :],
                                         start=(i == 0), stop=(i == KT - 1))
                    vo = st.tile([128, OC], BF16, name="st")
                    nc.scalar.copy(vo[:], pv[:])
                    tt = pos0 // 128 + sub
                    nc.gpsimd.dma_start(vsp[b][tt, :, :], vo[:])

        # ====== Phase 2+3: attention (h-outer), AllToAll, o_proj passes ======
        with ExitStack() as p2:
            vbp = p2.enter_context(tc.tile_pool(name="vb", bufs=2))
            ep = p2.enter_context(tc.tile_pool(name="ep", bufs=4))
            pvc = p2.enter_context(tc.tile_pool(name="pvc", bufs=4))
            trp = p2.enter_context(tc.tile_pool(name="tr", bufs=6))
            rc = p2.enter_context(tc.tile_pool(name="rc", bufs=4))
            ao = p2.enter_context(tc.tile_pool(name="ao", bufs=3))
            rhp = p2.enter_context(tc.tile_pool(name="rh", bufs=1))
            wop = p2.enter_context(tc.tile_pool(name="wo", bufs=6))
            oac = p2.enter_context(tc.tile_pool(name="oac", bufs=1))
            osb = p2.enter_context(tc.tile_pool(name="osb", bufs=4))
            ps_s = p2.enter_context(tc.tile_pool(name="ps_s", bufs=2, space="PSUM"))
            ps_pv = p2.enter_context(tc.tile_pool(name="ps_pv", bufs=2, space="PSUM"))
            ps_o = p2.enter_context(tc.tile_pool(name="ps_o", bufs=2, space="PSUM"))

            # v loads on the scalar ring, split so the first PV's kt tiles
            # land first; the wot/sync ring stays free for o_proj weights
            vb = [vbp.tile([128, LT, OC], BF16, name="vb") for _ in range(B)]
            for b in range(B):
                vsrc = vsp[b].ap().transpose([1, 0, 2])
                nc.scalar.dma_start(vb[b][:, 0:4, :], vsrc[:, 0:4, :])
                nc.scalar.dma_start(vb[b][:, 4:LT, :], vsrc[:, 4:LT, :])

            rh = rhp.tile([128, KT, SH], BF16, name="rh")
            rh4 = rh[:].rearrange("p (j f) t -> p j f t", f=4)   # [128,8,4,SH]
            out_acc = oac.tile([128, D // 128, SH], F32, name="oac")

            def attn_slot(h, b):
                    if h == 0 and b == 0:
                        q_sb, k_sb = q_pre, k_pre
                    else:
                        q_sb = qk.tile([128, L], BF16, name="q")
                        nc.scalar.dma_start(q_sb[:], qsp[b][h, :, :])
                        k_sb = qk.tile([128, L], BF16, name="k")
                        nc.scalar.dma_start(k_sb[:], ksp[b][h, :, :])
                    for half in range(2):
                        q0 = half * 1024
                        pvs = [ps_pv.tile([128, 512], F32, name="ps_pv")
                               for _ in range(2)]
                        tree = []          # bf16 pairwise row-sum tree
                        for kt in range(LT):
                            s_ps = ps_s.tile([128, 1024], F32, name="ps_s")
                            nc.tensor.matmul(s_ps[:, 0:512],
                                             k_sb[:, kt * 128:(kt + 1) * 128],
                                             q_sb[:, q0:q0 + 512],
                                             start=True, stop=True)
                            nc.tensor.matmul(s_ps[:, 512:1024],
                                             k_sb[:, kt * 128:(kt + 1) * 128],
                                             q_sb[:, q0 + 512:q0 + 1024],
                                             start=True, stop=True)
                            e_t = ep.tile([128, 1024], BF16, name="ep")
                            nc.scalar.activation(e_t[:], s_ps[:], EXP_F, scale=SCALE)
                            first, last = (kt == 0), (kt == LT - 1)
                            for c in range(2):
                                nc.tensor.matmul(pvs[c][:],
                                                 vb[b][:, kt, h * 128:(h + 1) * 128],
                                                 e_t[:, c * 512:(c + 1) * 512],
                                                 start=first, stop=last)
                            node = (0, e_t)
                            while tree and tree[-1][0] == node[0]:
                                prev = tree.pop()
                                nt = trp.tile([128, 1024], BF16, name="tr")
                                nc.vector.tensor_add(nt[:], prev[1][:], node[1][:])
                                node = (node[0] + 1, nt)
                            tree.append(node)
                        assert len(tree) == 1
                        root = tree[0][1]
                        # drain pv psums to SBUF so next half's MMs start now
                        pvcs = []
                        for c in range(2):
                            pc = pvc.tile([128, 512], F32, name="pvc")
                            nc.vector.tensor_copy(pc[:], pvs[c][:])
                            pvcs.append(pc)
                        # partition-reduce the row-sum tree root (pv slots free)
                        rts = [ps_pv.tile([128, 512], F32, name="ps_pv")
                               for _ in range(2)]
                        for c in range(2):
                            nc.tensor.matmul(rts[c][:], ones_sb[:],
                                             root[:, c * 512:(c + 1) * 512],
                                             start=True, stop=True)
                        for c in range(2):
                            rec = rc.tile([128, 512], F32, name="rc")
                            nc.vector.reciprocal_approx_fast(out=rec[:],
                                                             in_=rts[c][:])
                            at = ao.tile([128, 512], BF16, name="ao")
                            nc.vector.tensor_mul(at[:], pvcs[c][:], rec[:])
                            ci = half * 2 + c
                            nc.gpsimd.dma_start(
                                a2a_in[b][h][2 * ci, :, :], at[:, 0:SHB])
                            nc.gpsimd.dma_start(
                                a2a_in[b][h][2 * ci + 1, :, :], at[:, SHB:2 * SHB])
                    nc.gpsimd.collective_compute(
                        "AllToAll", mybir.AluOpType.bypass,
                        replica_groups=[list(range(NCORES))],
                        ins=[a2a_in[b][h].ap().opt()],
                        outs=[a2a_out[b][h].ap().opt()],
                    )
                    nc.scalar.dma_start(
                        rh4[:, :, h, b * SHB:(b + 1) * SHB],
                        a2a_out[b][h].ap().transpose([1, 0, 2]))

            def oproj_pass(h, c0, c1):
                # contract the 8 global heads {4j+h} over token cols [c0,c1)
                n = c1 - c0
                for ot in range(D // 128):
                    wot = wop.tile([128, NCORES, 128], BF16, name="wo")
                    nc.sync.dma_start(wot[:], wo[ot, :, h, :, :])
                    po = ps_o.tile([128, SH], F32, name="ps_o")
                    for j in range(NCORES):
                        nc.tensor.matmul(po[:, 0:n], wot[:, j, :],
                                         rh[:, 4 * j + h, c0:c1],
                                         start=(j == 0), stop=(j == NCORES - 1))
                    if h == 0:
                        nc.vector.tensor_copy(out_acc[:, ot, c0:c1], po[:, 0:n])
                    elif h < NH - 1:
                        nc.vector.tensor_add(out_acc[:, ot, c0:c1], po[:, 0:n],
                                             out_acc[:, ot, c0:c1])
                    else:
                        o_sb = osb.tile([128, SH], F32, name="osb")
                        nc.vector.tensor_add(o_sb[:, 0:n], po[:, 0:n],
                                             out_acc[:, ot, c0:c1])
                        nc.gpsimd.dma_start(out[ot * 128:(ot + 1) * 128, c0:c1],
                                            o_sb[:, 0:n])

            # slot order with lagged passes: each pass consumes its head's
            # AllToAlls at least one b1-slot (~40us) after the last one
            # fires, tolerating cross-core skew; P2 is held until after the
            # final slot so its 67us of matmuls cover the last AllToAll's
            # latency before P3 needs it
            attn_slot(0, 0); attn_slot(0, 1); attn_slot(1, 0); attn_slot(1, 1)
            oproj_pass(0, 0, SH)
            attn_slot(2, 0); attn_slot(2, 1)
            oproj_pass(1, 0, SH)
            attn_slot(3, 0); attn_slot(3, 1)
            oproj_pass(2, 0, SH)
            oproj_pass(3, 0, SH)

    nc.compile()
    return nc


def _qk_row_perm():
    # local row order: [h0re|h1re],[h0im|h1im],[h2re|h3re],[h2im|h3im]
    rows = []
    for pr in range(NH // 2):
        ha, hb = 2 * pr, 2 * pr + 1
        rows += [ha * HD + 2 * i for i in range(HD // 2)]
        rows += [hb * HD + 2 * i for i in range(HD // 2)]
        rows += [ha * HD + 2 * i + 1 for i in range(HD // 2)]
        rows += [hb * HD + 2 * i + 1 for i in range(HD // 2)]
    return np.array(rows)


def _prep_inputs(x, freqs_cos, freqs_sin, Wq, Wk, Wv, Wo):
    x = np.asarray(x, np.float32).reshape(T, D)
    Wq, Wk, Wv, Wo = (np.asarray(w, np.float32) for w in (Wq, Wk, Wv, Wo))
    fc = np.asarray(freqs_cos, np.float32)
    fs = np.asarray(freqs_sin, np.float32)

    # shared tensors
    xT = np.ascontiguousarray(
        x.reshape(T, KT, 128).transpose(1, 2, 0)).astype(NPBF16)        # [KT,128,T]
    # wo[ot, p, h, j, o] = Wo[ot*128+o, (4j+h)*128+p]
    woh = np.ascontiguousarray(
        Wo.reshape(D // 128, 128, NCORES, NH, 128).transpose(0, 4, 3, 2, 1)
    ).astype(NPBF16)
    csh = np.ascontiguousarray(np.concatenate([fc.T, fc.T], 0))          # [128,L]
    snh = np.ascontiguousarray(np.concatenate([fs.T, fs.T], 0))
    ones = np.ones([128, 128], NPBF16)

    perm = _qk_row_perm()
    in_maps = []
    for i in range(NCORES):
        rows = slice(OC * i, OC * (i + 1))
        wqi = Wq[rows][perm]                                             # [512, D]
        wki = Wk[rows][perm]
        wqh = np.ascontiguousarray(
            wqi.reshape(NH, 128, KT, 128).transpose(3, 0, 2, 1)).astype(NPBF16)
        wkh = np.ascontiguousarray(
            wki.reshape(NH, 128, KT, 128).transpose(3, 0, 2, 1)).astype(NPBF16)
        wvh = np.ascontiguousarray(
            Wv[rows].reshape(OC, KT, 128).transpose(2, 1, 0)).astype(NPBF16)
        in_maps.append({
            "xT": xT, "wq": wqh, "wk": wkh, "wv": wvh, "wo": woh,
            "cs": csh, "sn": snh, "ones": ones,
        })
    return in_maps


_NC_CACHE = None


def _get_nc():
    global _NC_CACHE
    if _NC_CACHE is None:
        _NC_CACHE = build_nc()
    return _NC_CACHE


def _run(in_maps, trace=False):
    nc = _get_nc()
    res = bass_utils.run_bass_kernel_spmd(
        nc, in_maps, core_ids=list(range(NCORES)), trace=trace)
    return res


def _assemble(results):
    out = np.empty((B, L, D), np.float32)
    for i in range(NCORES):
        o = results[i]["out"]                       # [D, SH] f32
        for b in range(B):
            out[b, SHB * i:SHB * (i + 1), :] = o[:, b * SHB:(b + 1) * SHB].T
    return out


def kernel(x, freqs_cos, freqs_sin, Wq, Wk, Wv, Wo):
    in_maps = _prep_inputs(x, freqs_cos, freqs_sin, Wq, Wk, Wv, Wo)
    res = _run(in_maps, trace=False)
    return _assemble(res.results)


# revision 25
# speedup vs baseline: 1.1491x; 1.0310x over previous
"""Distributed multi-head attention (B=2, L=2048, D=4096, H=32) on 8 TRN2 NeuronCores.

Strategy: tensor-parallel over heads (4 heads/core) for QKV+attention, then an
AllToAll that trades head-dims for token-slices so o_proj is token-sharded
(each core computes out[:, its 512 tokens] with the full Wo) — the AllToAll
moves 4 MB/core instead of the 64 MB/core an output AllReduce would.

All matmuls run in bf16 on the TensorEngine (f32 PSUM accumulation).
Host-side prep: transpose/permute/tile weights and x into DMA-friendly
partition-major layouts, pre-cast to bf16. Host post: concatenate the 8
token-shards and transpose.

RoPE trick: Q/K output columns are permuted host-side (per head: even dims
then odd dims, pairs of heads interleaved into 128-row tiles) so the rotation
becomes full-width [128, t] vector ops with no partition-pair shuffles.

Schedule: startup DMAs are split across rings in first-use order (wq as 4
per-head slices + wk/wv on the scalar ring; x-group tiles as 4 slice-DMAs on
sync, slice 3 first with all kt-chains rotated [24..31,0..23] so refills are
just-in-time; cos/sin on scalar so the sync DMA counter the first matmul
waits on covers only x).  Attention loops h-outer/batch-inner; (h0,b0) q/k
prefetch on the gpsimd ring at g==4 so the scheduler hoists early b0 slots
into the QKV phase's exp-shadow.  o_proj runs as 4 per-local-head passes (Wo
pre-grouped host-side by local head) accumulating in SBUF f32 on the
VectorEngine; each pass is issued >=1 b1-slot after its last AllToAll fires
(cross-core skew tolerance) and pass 2 is held until after the final slot so
its 67us of matmuls cover the last AllToAll's latency before pass 3 — the
post-attention tail is one 67us pass instead of 250us.  rh/v loads ride the
scalar ring so the sync ring streams o_proj weights uninterrupted.
Measured 1.459ms on silicon (PE busy 1.357ms vs ~1.35ms bf16 floor at the
13/16 board clock), rel-err 6.4e-3; baseline was 1.503ms.
"""

import sys

if "/opt/trn_rl_repo" not in sys.path:
    sys.path.insert(0, "/opt/trn_rl_repo")

from contextlib import ExitStack

import ml_dtypes
import numpy as np

import concourse.bass as bass
import concourse.tile as tile
from concourse import bacc, mybir
from concourse import bass_utils

BF16 = mybir.dt.bfloat16
F32 = mybir.dt.float32
NPBF16 = ml_dtypes.bfloat16

NCORES = 8
B, L, D, H, HD = 2, 2048, 4096, 32, 128
T = B * L              # 4096 global tokens
NH = H // NCORES       # 4 heads per core
OC = NH * HD           # 512 projection dims per core
KT = D // 128          # 32 contraction tiles over D
LT = L // 128          # 16 key tiles per batch
TG = 512               # phase-1 token-group width
NG = T // TG           # 8 groups
SH = T // NCORES       # 512 output tokens per core
SHB = SH // B          # 256 per batch
SCALE = 1.0 / float(np.sqrt(HD))

EXP_F = mybir.ActivationFunctionType.Exp


def build_nc():
    nc = bacc.Bacc("TRN2", target_bir_lowering=False, debug=False,
                   num_devices=NCORES)

    # ---- I/O (per-core shards, host-pretiled, bf16) ----
    # x grouped [NG, 128, KT, TG]: each (group, partition) row is KT*TG
    # contiguous, so the per-group slice DMAs move 8KB lines per partition
    xT = nc.dram_tensor("xT", [NG, 128, KT, TG], BF16, kind="ExternalInput")
    wq = nc.dram_tensor("wq", [128, NH, KT, 128], BF16, kind="ExternalInput")
    wk = nc.dram_tensor("wk", [128, NH, KT, 128], BF16, kind="ExternalInput")
    wv = nc.dram_tensor("wv", [128, KT, OC], BF16, kind="ExternalInput")
    # wo grouped by local head: wo[ot, p, h, j, o] = Wo[ot*128+o, (4j+h)*128+p]
    wo = nc.dram_tensor("wo", [D // 128, 128, NH, NCORES, 128], BF16,
                        kind="ExternalInput")
    cs = nc.dram_tensor("cs", [128, L], F32, kind="ExternalInput")
    sn = nc.dram_tensor("sn", [128, L], F32, kind="ExternalInput")
    ones = nc.dram_tensor("ones", [128, 128], BF16, kind="ExternalInput")
    out = nc.dram_tensor("out", [D, SH], F32, kind="ExternalOutput")

    # ---- internal DRAM (spills + collective bounce) ----
    qsp = [nc.dram_tensor(f"qsp{b}", [NH, 128, L], BF16) for b in range(B)]
    ksp = [nc.dram_tensor(f"ksp{b}", [NH, 128, L], BF16) for b in range(B)]
    vsp = [nc.dram_tensor(f"vsp{b}", [LT, 128, OC], BF16) for b in range(B)]
    # AllToAll split by (batch, head): [shard, HD dims, SHB tokens]
    a2a_in = [[nc.dram_tensor(f"a2ai{b}_{h}", [NCORES, HD, SHB], BF16)
               for h in range(NH)] for b in range(B)]
    a2a_out = [[nc.dram_tensor(f"a2ao{b}_{h}", [NCORES, HD, SHB], BF16)
                for h in range(NH)] for b in range(B)]

    with tile.TileContext(nc) as tc, ExitStack() as ctx:
        singles = ctx.enter_context(tc.tile_pool(name="singles", bufs=1))
        ones_sb = singles.tile([128, 128], BF16, name="ones")
        nc.gpsimd.dma_start(ones_sb[:], ones[:, :])

        # pools that span the phase boundary (prefetch targets)
        qk = ctx.enter_context(tc.tile_pool(name="qk", bufs=2))
        q_pre = qk.tile([128, L], BF16, name="q")
        k_pre = qk.tile([128, L], BF16, name="k")

        # ================= Phase 1: QKV projections + RoPE =================
        with ExitStack() as p1:
            wpool = p1.enter_context(tc.tile_pool(name="w", bufs=1))
            # weights stream on the scalar ring in first-use order; wq is
            # filled by two slice-DMAs so pr=0 starts after the first 2MB
            wq_sb = wpool.tile([128, NH, KT, 128], BF16, name="wq")
            for hh in range(NH):
                nc.scalar.dma_start(wq_sb[:, hh:hh + 1, :, :],
                                    wq[:, hh:hh + 1, :, :])
            wk_sb = wpool.tile([128, NH, KT, 128], BF16, name="wk")
            nc.scalar.dma_start(wk_sb[:], wk[:, :, :, :])
            wv_sb = wpool.tile([128, KT, OC], BF16, name="wv")
            nc.scalar.dma_start(wv_sb[:], wv[:, :, :])

            xpool = p1.enter_context(tc.tile_pool(name="xg", bufs=2))
            cpool = p1.enter_context(tc.tile_pool(name="csg", bufs=2))
            tmp = p1.enter_context(tc.tile_pool(name="tmp", bufs=8))
            st = p1.enter_context(tc.tile_pool(name="st", bufs=6))
            ps1 = p1.enter_context(tc.tile_pool(name="ps1", bufs=6, space="PSUM"))

            for g in range(NG):
                b = g // (NG // B)
                pos0 = (g % (NG // B)) * TG          # position within batch
                xg = xpool.tile([128, KT, TG], BF16, name="xg")
                # 4 slice loads, slice 3 first: all kt-chains consume in
                # rotated order [24..31, 0..23] so refills are just-in-time
                xsrc = xT[g, :, :, :]
                nc.sync.dma_start(xg[:, 24:32, :], xsrc[:, 24:32, :])
                # cos/sin ride the scalar ring: keeps the sync ring's DMA
                # counter (which the first matmul of the group waits on)
                # covering only the x slices
                csg = cpool.tile([128, TG], F32, name="csg")
                nc.scalar.dma_start(csg[:], cs[:, pos0:pos0 + TG])
                sng = cpool.tile([128, TG], F32, name="sng")
                nc.scalar.dma_start(sng[:], sn[:, pos0:pos0 + TG])
                for s in range(3):
                    nc.sync.dma_start(xg[:, 8 * s:8 * (s + 1), :],
                                      xsrc[:, 8 * s:8 * (s + 1), :])

                if g == 4:
                    # b0 spills are complete: prefetch (h0,b0) q/k on the
                    # gpsimd ring so attention starts with no gap
                    nc.gpsimd.dma_start(q_pre[:], qsp[0][0, :, :])
                    nc.gpsimd.dma_start(k_pre[:], ksp[0][0, :, :])

                # Q and K with fused RoPE (kt order rotated, see above)
                kto = list(range(24, 32)) + list(range(24))
                for wsb, sp in ((wq_sb, qsp[b]), (wk_sb, ksp[b])):
                    for pr in range(NH // 2):
                        p_re = ps1.tile([128, TG], F32, name="ps1")
                        p_im = ps1.tile([128, TG], F32, name="ps1")
                        for i, kt in enumerate(kto):
                            nc.tensor.matmul(p_re[:], wsb[:, 2 * pr, kt, :],
                                             xg[:, kt, :],
                                             start=(i == 0), stop=(i == KT - 1))
                        for i, kt in enumerate(kto):
                            nc.tensor.matmul(p_im[:], wsb[:, 2 * pr + 1, kt, :],
                                             xg[:, kt, :],
                                             start=(i == 0), stop=(i == KT - 1))
                        t1 = tmp.tile([128, TG], F32, name="tmp")
                        t2 = tmp.tile([128, TG], F32, name="tmp")
                        t3 = tmp.tile([128, TG], F32, name="tmp")
                        t4 = tmp.tile([128, TG], F32, name="tmp")
                        o_re = st.tile([128, TG], BF16, name="st")
                        o_im = st.tile([128, TG], BF16, name="st")
                        nc.vector.tensor_mul(t1[:], p_re[:], csg[:])
                        nc.vector.tensor_mul(t2[:], p_im[:], sng[:])
                        nc.vector.tensor_sub(o_re[:], t1[:], t2[:])
                        nc.vector.tensor_mul(t3[:], p_re[:], sng[:])
                        nc.vector.tensor_mul(t4[:], p_im[:], csg[:])
                        nc.vector.tensor_add(o_im[:], t3[:], t4[:])
                        ha, hb = 2 * pr, 2 * pr + 1
                        nc.gpsimd.dma_start(sp[ha, 0:64, pos0:pos0 + TG],
                                            o_re[0:64, :])
                        nc.gpsimd.dma_start(sp[hb, 0:64, pos0:pos0 + TG],
                                            o_re[64:128, :])
                        nc.gpsimd.dma_start(sp[ha, 64:128, pos0:pos0 + TG],
                                            o_im[0:64, :])
                        nc.gpsimd.dma_start(sp[hb, 64:128, pos0:pos0 + TG],
                                            o_im[64:128, :])

                # V (layout [t, oc])
                for sub in range(TG // 128):
                    pv = ps1.tile([128, OC], F32, name="ps1")
                    for i, kt in enumerate(kto):
                        nc.tensor.matmul(pv[:], xg[:, kt, sub * 128:(sub + 1) * 128],
                                         wv_sb[:, kt, :],
                                         start=(i == 0), stop=(i == KT - 1))
                    vo = st.tile([128, OC], BF16, name="st")
                    nc.scalar.copy(vo[:], pv[:])
                    tt = pos0 // 128 + sub
                    nc.gpsimd.dma_start(vsp[b][tt, :, :], vo[:])

        # ====== Phase 2+3: attention (h-outer), AllToAll, o_proj passes ======
        with ExitStack() as p2:
            vbp = p2.enter_context(tc.tile_pool(name="vb", bufs=2))
            ep = p2.enter_context(tc.tile_pool(name="ep", bufs=4))
            pvc = p2.enter_context(tc.tile_pool(name="pvc", bufs=4))
            trp = p2.enter_context(tc.tile_pool(name="tr", bufs=6))
            rc = p2.enter_context(tc.tile_pool(name="rc", bufs=4))
            ao = p2.enter_context(tc.tile_pool(name="ao", bufs=3))
            rhp = p2.enter_context(tc.tile_pool(name="rh", bufs=1))
            wop = p2.enter_context(tc.tile_pool(name="wo", bufs=6))
            oac = p2.enter_context(tc.tile_pool(name="oac", bufs=1))
            osb = p2.enter_context(tc.tile_pool(name="osb", bufs=4))
            ps_s = p2.enter_context(tc.tile_pool(name="ps_s", bufs=2, space="PSUM"))
            ps_pv = p2.enter_context(tc.tile_pool(name="ps_pv", bufs=2, space="PSUM"))
            ps_o = p2.enter_context(tc.tile_pool(name="ps_o", bufs=2, space="PSUM"))

            # v loads on the scalar ring, split so the first PV's kt tiles
            # land first; the wot/sync ring stays free for o_proj weights
            vb = [vbp.tile([128, LT, OC], BF16, name="vb") for _ in range(B)]
            for b in range(B):
                vsrc = vsp[b].ap().transpose([1, 0, 2])
                nc.scalar.dma_start(vb[b][:, 0:4, :], vsrc[:, 0:4, :])
                nc.scalar.dma_start(vb[b][:, 4:LT, :], vsrc[:, 4:LT, :])

            rh = rhp.tile([128, KT, SH], BF16, name="rh")
            rh4 = rh[:].rearrange("p (j f) t -> p j f t", f=4)   # [128,8,4,SH]
            out_acc = oac.tile([128, D // 128, SH], F32, name="oac")

            def attn_slot(h, b):
                    if h == 0 and b == 0:
                        q_sb, k_sb = q_pre, k_pre
                    else:
                        q_sb = qk.tile([128, L], BF16, name="q")
                        nc.scalar.dma_start(q_sb[:], qsp[b][h, :, :])
                        k_sb = qk.tile([128, L], BF16, name="k")
                        nc.scalar.dma_start(k_sb[:], ksp[b][h, :, :])
                    for half in range(2):
                        q0 = half * 1024
                        pvs = [ps_pv.tile([128, 512], F32, name="ps_pv")
                               for _ in range(2)]
                        tree = []          # bf16 pairwise row-sum tree
                        for kt in range(LT):
                            s_ps = ps_s.tile([128, 1024], F32, name="ps_s")
                            nc.tensor.matmul(s_ps[:, 0:512],
                                             k_sb[:, kt * 128:(kt + 1) * 128],
                                             q_sb[:, q0:q0 + 512],
                                             start=True, stop=True)
                            nc.tensor.matmul(s_ps[:, 512:1024],
                                             k_sb[:, kt * 128:(kt + 1) * 128],
                                             q_sb[:, q0 + 512:q0 + 1024],
                                             start=True, stop=True)
                            e_t = ep.tile([128, 1024], BF16, name="ep")
                            nc.scalar.activation(e_t[:], s_ps[:], EXP_F, scale=SCALE)
                            first, last = (kt == 0), (kt == LT - 1)
                            for c in range(2):
                                nc.tensor.matmul(pvs[c][:],
                                                 vb[b][:, kt, h * 128:(h + 1) * 128],
                                                 e_t[:, c * 512:(c + 1) * 512],
                                                 start=first, stop=last)
                            node = (0, e_t)
                            while tree and tree[-1][0] == node[0]:
                                prev = tree.pop()
                                nt = trp.tile([128, 1024], BF16, name="tr")
                                nc.vector.tensor_add(nt[:], prev[1][:], node[1][:])
                                node = (node[0] + 1, nt)
                            tree.append(node)
                        assert len(tree) == 1
                        root = tree[0][1]
                        # drain pv psums to SBUF so next half's MMs start now
                        pvcs = []
                        for c in range(2):
                            pc = pvc.tile([128, 512], F32, name="pvc")
                            nc.vector.tensor_copy(pc[:], pvs[c][:])
                            pvcs.append(pc)
                        # partition-reduce the row-sum tree root (pv slots free)
                        rts = [ps_pv.tile([128, 512], F32, name="ps_pv")
                               for _ in range(2)]
                        for c in range(2):
                            nc.tensor.matmul(rts[c][:], ones_sb[:],
                                             root[:, c * 512:(c + 1) * 512],
                                             start=True, stop=True)
                        for c in range(2):
                            rec = rc.tile([128, 512], F32, name="rc")
                            nc.vector.reciprocal_approx_fast(out=rec[:],
                                                             in_=rts[c][:])
                            at = ao.tile([128, 512], BF16, name="ao")
                            nc.vector.tensor_mul(at[:], pvcs[c][:], rec[:])
                            ci = half * 2 + c
                            nc.gpsimd.dma_start(
                                a2a_in[b][h][2 * ci, :, :], at[:, 0:SHB])
                            nc.gpsimd.dma_start(
                                a2a_in[b][h][2 * ci + 1, :, :], at[:, SHB:2 * SHB])
                    nc.gpsimd.collective_compute(
                        "AllToAll", mybir.AluOpType.bypass,
                        replica_groups=[list(range(NCORES))],
                        ins=[a2a_in[b][h].ap().opt()],
                        outs=[a2a_out[b][h].ap().opt()],
                    )
                    # gpsimd ring: a stalled rh descriptor (waiting on its
                    # collective) must not block the scalar ring's q/k
                    # prefetches or the sync ring's o_proj weight stream
                    nc.gpsimd.dma_start(
                        rh4[:, :, h, b * SHB:(b + 1) * SHB],
                        a2a_out[b][h].ap().transpose([1, 0, 2]))

            def oproj_pass(h, c0, c1):
                # contract the 8 global heads {4j+h} over token cols [c0,c1)
                n = c1 - c0
                for ot in range(D // 128):
                    wot = wop.tile([128, NCORES, 128], BF16, name="wo")
                    nc.sync.dma_start(wot[:], wo[ot, :, h, :, :])
                    po = ps_o.tile([128, SH], F32, name="ps_o")
                    for j in range(NCORES):
                        nc.tensor.matmul(po[:, 0:n], wot[:, j, :],
                                         rh[:, 4 * j + h, c0:c1],
                                         start=(j == 0), stop=(j == NCORES - 1))
                    if h == 0:
                        nc.vector.tensor_copy(out_acc[:, ot, c0:c1], po[:, 0:n])
                    elif h < NH - 1:
                        nc.vector.tensor_add(out_acc[:, ot, c0:c1], po[:, 0:n],
                                             out_acc[:, ot, c0:c1])
                    else:
                        o_sb = osb.tile([128, SH], F32, name="osb")
                        nc.vector.tensor_add(o_sb[:, 0:n], po[:, 0:n],
                                             out_acc[:, ot, c0:c1])
                        nc.gpsimd.dma_start(out[ot * 128:(ot + 1) * 128, c0:c1],
                                            o_sb[:, 0:n])

            # slot order with lagged passes: each pass consumes its head's
            # AllToAlls at least one b1-slot (~40us) after the last one
            # fires, tolerating cross-core skew; P2 is held until after the
            # final slot so its 67us of matmuls cover the last AllToAll's
            # latency before P3 needs it
            attn_slot(0, 0); attn_slot(0, 1); attn_slot(1, 0); attn_slot(1, 1)
            attn_slot(2, 0)
            oproj_pass(0, 0, SH)
            attn_slot(2, 1)
            oproj_pass(1, 0, SH)
            attn_slot(3, 0); attn_slot(3, 1)
            oproj_pass(2, 0, SH)
            oproj_pass(3, 0, SH)

    nc.compile()
    return nc


def _qk_row_perm():
    # local row order: [h0re|h1re],[h0im|h1im],[h2re|h3re],[h2im|h3im]
    rows = []
    for pr in range(NH // 2):
        ha, hb = 2 * pr, 2 * pr + 1
        rows += [ha * HD + 2 * i for i in range(HD // 2)]
        rows += [hb * HD + 2 * i for i in range(HD // 2)]
        rows += [ha * HD + 2 * i + 1 for i in range(HD // 2)]
        rows += [hb * HD + 2 * i + 1 for i in range(HD // 2)]
    return np.array(rows)


def _prep_inputs(x, freqs_cos, freqs_sin, Wq, Wk, Wv, Wo):
    x = np.asarray(x, np.float32).reshape(T, D)
    Wq, Wk, Wv, Wo = (np.asarray(w, np.float32) for w in (Wq, Wk, Wv, Wo))
    fc = np.asarray(freqs_cos, np.float32)
    fs = np.asarray(freqs_sin, np.float32)

    # shared tensors
    xT = np.ascontiguousarray(
        x.reshape(NG, TG, KT, 128).transpose(0, 3, 2, 1)).astype(NPBF16)  # [NG,128,KT,TG]
    # wo[ot, p, h, j, o] = Wo[ot*128+o, (4j+h)*128+p]
    woh = np.ascontiguousarray(
        Wo.reshape(D // 128, 128, NCORES, NH, 128).transpose(0, 4, 3, 2, 1)
    ).astype(NPBF16)
    csh = np.ascontiguousarray(np.concatenate([fc.T, fc.T], 0))          # [128,L]
    snh = np.ascontiguousarray(np.concatenate([fs.T, fs.T], 0))
    ones = np.ones([128, 128], NPBF16)

    perm = _qk_row_perm()
    in_maps = []
    for i in range(NCORES):
        rows = slice(OC * i, OC * (i + 1))
        wqi = Wq[rows][perm]                                             # [512, D]
        wki = Wk[rows][perm]
        wqh = np.ascontiguousarray(
            wqi.reshape(NH, 128, KT, 128).transpose(3, 0, 2, 1)).astype(NPBF16)
        wkh = np.ascontiguousarray(
            wki.reshape(NH, 128, KT, 128).transpose(3, 0, 2, 1)).astype(NPBF16)
        wvh = np.ascontiguousarray(
            Wv[rows].reshape(OC, KT, 128).transpose(2, 1, 0)).astype(NPBF16)
        in_maps.append({
            "xT": xT, "wq": wqh, "wk": wkh, "wv": wvh, "wo": woh,
            "cs": csh, "sn": snh, "ones": ones,
        })
    return in_maps


_NC_CACHE = None


def _get_nc():
    global _NC_CACHE
    if _NC_CACHE is None:
        _NC_CACHE = build_nc()
    return _NC_CACHE


def _run(in_maps, trace=False):
    nc = _get_nc()
    res = bass_utils.run_bass_kernel_spmd(
        nc, in_maps, core_ids=list(range(NCORES)), trace=trace)
    return res


def _assemble(results):
    out = np.empty((B, L, D), np.float32)
    for i in range(NCORES):
        o = results[i]["out"]                       # [D, SH] f32
        for b in range(B):
            out[b, SHB * i:SHB * (i + 1), :] = o[:, b * SHB:(b + 1) * SHB].T
    return out


def kernel(x, freqs_cos, freqs_sin, Wq, Wk, Wv, Wo):
    in_maps = _prep_inputs(x, freqs_cos, freqs_sin, Wq, Wk, Wv, Wo)
    res = _run(in_maps, trace=False)
    return _assemble(res.results)
